# revision 123
# baseline (speedup 1.0000x reference)
"""Causal self-attention (B=2, L=2048, E=768, H=12) on 8 trn2 NeuronCores.

Sharding: data parallel over B (cores 0-3 -> b=0, cores 4-7 -> b=1), tensor
parallel over heads (each core owns 3 heads).  Per core:
  - all-bf16 operands; q/k projections in transposed [d, L] layout,
  - scores kept TRANSPOSED S^T [keys, queries]; heads 0/1 as two K=64 PE
    row-tile matmuls (partition bases 0/64); head 2 pairs adjacent key
    blocks the same way using replicated q2/k2 copies,
  - numerator stationary is [V_h | ones] so PSUM rows 64:128 accumulate the
    softmax denominator replicated across 64 partitions; normalization is
    copy + fast approx-reciprocal + multiply on DVE,
  - causal diagonal masking via a bf16 multiply on the exp output (DVE),
  - query chunks processed 0..3; the attention stream (scores -> exp ->
    mask -> numerator) is ACT-limited, so ALL projection work is emitted as
    interleaved PE "filler" units between attention blocks: chunk j carries
    the q/k/v projections of chunk j+1 and the out-projection of chunk j-1,
  - per chunk: y (3 heads) is published (h01 rows early, right after their
    normalize) and AllGathered (bf16, 192KB -> 768KB, one per chunk on the
    otherwise-empty gpsimd queue); every core computes the 12-head
    out-projection for its 192 output columns as 6 K=128 head-PAIR matmuls
    (+64-col half), tanh+bias on ACT, bf16 output,
  - PSUM: scores 2x[128,2,512] (4 banks) + numerators (3) + filler/outproj
    ring (1); startup wire order: wqk slots 0/1 + chunk-0 x first,
  - persistent tiles (q/k/v/y, the 'ones' memset, and the load-once consts
    cf32 + wo) are hoisted out of the body; in unrolled builds each
    iteration's tail out-projection is DEFERRED into the next iteration's
    chunk-0 filler drain so a pending AllGather never head-of-line-blocks
    the next iteration's projections on the in-order PE queue (requires the
    deferred units to read only load-once tiles, else reload-WARs deadlock
    the ACT queue).
HW timing (unrolled-with-collectives interleaved slope): ~100-106 us/iter;
TimelineSim (no-collective single body): 113.6 us.
"""
import hashlib
import os
import shutil

import numpy as np

import concourse.bacc as bacc
import concourse.mybir as mybir
import concourse.tile as tile
from concourse import bass_utils, bass2jax

F32 = mybir.dt.float32
BF16 = mybir.dt.bfloat16
AF = mybir.ActivationFunctionType

B, L, E, H, D = 2, 2048, 768, 12, 64
HPC = 3                      # heads per core
NC = 8
GROUPS = [[0, 1, 2, 3], [4, 5, 6, 7]]
EC = E // 128                # 6 embedding chunks
QC = L // 512                # 4 query chunks of 512
KB = L // 128                # 16 key blocks of 128

# ---------------------------------------------------------------------------
# NEFF compile memoization (same BIR -> same NEFF); safe, process-local.
_orig_compile = bass_utils.compile_bir_kernel
_CACHE_DIR = os.environ.get("NEFF_MEMO_DIR", "/tmp/neff_cache")


def _memo_compile(bir_json, tmpdir, neff_name="file.neff"):
    try:
        os.makedirs(_CACHE_DIR, exist_ok=True)
        key = hashlib.sha256(bir_json).hexdigest()[:24]
        cached = os.path.join(_CACHE_DIR, f"{key}.neff")
        if os.path.exists(cached):
            dst = os.path.join(tmpdir, neff_name)
            shutil.copy(cached, dst)
            return dst
        path = _orig_compile(bir_json, tmpdir, neff_name)
        shutil.copy(path, cached)
        return path
    except OSError:
        return _orig_compile(bir_json, tmpdir, neff_name)


bass_utils.compile_bir_kernel = _memo_compile
bass2jax.compile_bir_kernel = _memo_compile


# ---------------------------------------------------------------------------
def _make_pers(nc, pers, io):
    """Persistent tiles + iteration-invariant consts, allocated/loaded once
    per NEFF.  wo_t and cf32 MUST be load-once: a per-iteration reload would
    WAR against the deferred tail out-projection's reads and deadlock the
    in-order ACT queue (reload -> deferred tanh -> behind next exps)."""
    (xT, wqk, wv, wo, cbf_d, cf32_d, out_bt) = io
    qTp = pers.tile([128, L], BF16, name="qTp")    # h0 rows 0:64, h1 64:128
    kTp = pers.tile([128, L], BF16, name="kTp")
    q2rep = pers.tile([128, L], BF16, name="q2rep")  # h2 q replicated
    k2rep = pers.tile([128, L], BF16, name="k2rep")  # h2 k replicated
    v_t = pers.tile([128, KB, HPC, 128], BF16, name="v_t")  # [keys,kb,h,v|1]
    yTall = pers.tile([64, HPC, L], BF16, name="yTall")
    # once per NEFF, at the head of the gpsimd queue (before any collective
    # is enqueued, so nothing blocks); the Pool engine's DMA path runs in
    # parallel with the HWDGE wire, keeping these off the startup x path
    nc.gpsimd.memset(v_t[:, :, :, 64:128], 1.0)
    cf32 = pers.tile([128, 197], F32, name="cf32")  # bqk|bv|bo1|bo2
    nc.gpsimd.dma_start(out=cf32, in_=cf32_d.ap())
    wo_t = pers.tile([128, H // 2, 192], BF16, name="wo_t")
    nc.gpsimd.dma_start(out=wo_t, in_=wo.ap())
    return (qTp, kTp, q2rep, k2rep, v_t, yTall, cf32, wo_t)


def _emit_body(nc, tc, io, pools, pers_tiles, with_collective=True,
               defer_tail=False, pending_tail=None):
    (xT, wqk, wv, wo, cbf_d, cf32_d, out_bt) = io
    consts, pers, work, sc, num, po, dram = pools

    # ---- constant loads -------------------------------------------------
    # wire-order startup: wqk + chunk-0 x columns first (they gate the first
    # projections), then the small const blobs, then the rest of x.  The
    # gpsimd queue carries ONLY the collectives so an in-flight AllGather
    # never blocks the next iteration's loads.
    wqk_t = consts.tile([128, 3, EC, 128], BF16, name="wqk_t")
    nc.scalar.dma_start(out=wqk_t[:, 0:2], in_=wqk.ap()[:, 0:2])
    xt_t = consts.tile([128, EC, L], BF16, name="xt_t")
    xT_r = xT.ap().rearrange("(c p) m -> p c m", p=128)
    qs_eng = [nc.sync, nc.scalar]
    for c in range(EC):
        qs_eng[c % 2].dma_start(out=xt_t[:, c, 0:512], in_=xT_r[:, c, 0:512])
    nc.scalar.dma_start(out=wqk_t[:, 2:3], in_=wqk.ap()[:, 2:3])
    cbf = consts.tile([128, 768], BF16, name="cbf")   # tri2 | mh2
    nc.sync.dma_start(out=cbf, in_=cbf_d.ap())
    wv_t = consts.tile([128, EC, 192], BF16, name="wv_t")
    nc.scalar.dma_start(out=wv_t, in_=wv.ap())
    tri2_t = cbf[:, 0:256].rearrange("p (a b) -> p a b", a=2)
    mh2_t = cbf[:, 256:768].rearrange("p (a b) -> p a b", a=2)
    qi = 0
    for jj in range(1, QC):
        jsl = slice(512 * jj, 512 * jj + 512)
        for c in range(EC):
            qs_eng[qi % 2].dma_start(out=xt_t[:, c, jsl], in_=xT_r[:, c, jsl])
            qi += 1

    # ---- persistent tiles + load-once consts (shared across iterations) --
    (qTp, kTp, q2rep, k2rep, v_t, yTall, cf32, wo_t) = pers_tiles
    bqk_t = cf32[:, 0:3]
    bv_t = cf32[:, 3:195]
    bo1_t = cf32[:, 195:196]
    bo2_t = cf32[0:64, 196:197]

    ag_ins = [dram.tile([192, 512], BF16, name=f"ag_in{j}") for j in range(QC)]
    ag_outs = [dram.tile([768, 512], BF16, name=f"ag_out{j}")
               for j in range(QC)]

    # ---- projection / out-projection units (emitted interleaved) --------
    # slot 0 = [Wq_h0|Wq_h1], slot 1 = [Wk_h0|Wk_h1], slot 2 = [Wq_h2|Wk_h2]
    def _qk_epilogue(jj, slot, ps):
        jsl = slice(512 * jj, 512 * jj + 512)
        if slot == 0:
            nc.vector.tensor_scalar_add(out=qTp[:, jsl], in0=ps,
                                        scalar1=bqk_t[:, 0:1])
        elif slot == 1:
            nc.vector.tensor_scalar_add(out=kTp[:, jsl], in0=ps,
                                        scalar1=bqk_t[:, 1:2])
        else:
            nc.vector.tensor_scalar_add(out=q2rep[0:64, jsl], in0=ps[0:64],
                                        scalar1=bqk_t[0:64, 2:3])
            nc.vector.tensor_scalar_add(out=k2rep[64:128, jsl],
                                        in0=ps[64:128],
                                        scalar1=bqk_t[64:128, 2:3])
            nc.sync.dma_start(out=q2rep[64:128, jsl], in_=q2rep[0:64, jsl])
            nc.sync.dma_start(out=k2rep[0:64, jsl], in_=k2rep[64:128, jsl])

    def qkproj_unit(jj, slot, tag="po"):
        jsl = slice(512 * jj, 512 * jj + 512)
        ps = po.tile([128, 512], F32, tag=tag, name=f"ps_qk{slot}_{jj}")
        for c in range(EC):
            nc.tensor.matmul(ps, wqk_t[:, slot, c], xt_t[:, c, jsl],
                             start=(c == 0), stop=(c == EC - 1))
        _qk_epilogue(jj, slot, ps)

    def vproj_unit(lc, in_sc=False):
        pool_, tag_ = (sc, "sc") if in_sc else (po, "po")
        ps = pool_.tile([128, 192], F32, tag=tag_, name=f"ps_v{lc}")
        for c in range(EC):
            nc.tensor.matmul(ps, xt_t[:, c, 128 * lc:128 * lc + 128], wv_t[:, c],
                             start=(c == 0), stop=(c == EC - 1))
        nc.vector.tensor_add(v_t[:, lc, :, 0:64],
                             ps.rearrange("p (h d) -> p h d", h=HPC),
                             bv_t.rearrange("p (h d) -> p h d", h=HPC))

    def outproj_units(j):
        # head-PAIR layout: yAll [128, 6, 512], partitions 0:64 = head 2p,
        # 64:128 = head 2p+1 -> full K=128 contraction per matmul.
        jsl = slice(512 * j, 512 * j + 512)
        yAll = work.tile([128, H // 2, 512], BF16, tag="yall", name=f"yAll{j}")
        po_t = [None, None]
        src = ag_outs[j].rearrange("(hp p) q -> p hp q", p=128)

        def u_load():
            # per-pair DMAs (sync queue) so out-proj matmuls pipeline with
            # the gather read; the no-collective fallback mimics the same 6
            # DRAM reads (first one depends on the publish, as the AG would)
            if with_collective:
                for p in range(H // 2):
                    nc.sync.dma_start(out=yAll[:, p, :], in_=src[:, p, :])
            else:
                nc.sync.dma_start(out=yAll[:, 0, :], in_=ag_ins[j][0:128, :])
                for p in range(1, H // 2):
                    nc.sync.dma_start(out=yAll[:, p, :], in_=src[:, p, :])

        def u_mm1():
            po_t[0] = po.tile([128, 512], F32, tag="po", name=f"po1_{j}")
            for p in range(H // 2):
                nc.tensor.matmul(po_t[0], wo_t[:, p, 0:128], yAll[:, p, :],
                                 start=(p == 0), stop=(p == H // 2 - 1))

        def u_act1():
            t1 = work.tile([128, 512], BF16, tag="t1", name=f"t1_{j}")
            nc.scalar.activation(t1, po_t[0], AF.Tanh, bias=bo1_t, scale=1.0)
            nc.sync.dma_start(out=out_bt.ap()[0:128, jsl], in_=t1)

        def u_mm2():
            po_t[1] = po.tile([64, 512], F32, tag="po", name=f"po2_{j}")
            for p in range(H // 2):
                nc.tensor.matmul(po_t[1], wo_t[:, p, 128:192], yAll[:, p, :],
                                 start=(p == 0), stop=(p == H // 2 - 1))

        def u_act2():
            t2 = work.tile([64, 512], BF16, tag="t2", name=f"t2_{j}")
            nc.scalar.activation(t2, po_t[1], AF.Tanh, bias=bo2_t, scale=1.0)
            nc.scalar.dma_start(out=out_bt.ap()[128:192, jsl], in_=t2)

        return [u_load, u_mm1, u_act1, u_mm2, u_act2]

    # ---- attention chunk with interleaved filler units ------------------
    def attn_chunk(j, fillers=()):
        fit = iter(fillers)

        def fill():
            f = next(fit, None)
            if f is not None:
                f()

        jsl = slice(512 * j, 512 * j + 512)
        pn01 = num.tile([128, 2, 512], F32, tag="pn01", name=f"pn01_{j}")
        pn2 = num.tile([128, 512], F32, tag="pn2", name=f"pn2_{j}")

        # heads 0/1: one key block per step, two concurrent row tiles
        def h01_block(kb):
            w0 = max(0, 128 * kb - 512 * j)
            w = 512 - w0
            qs = 512 * j + w0
            psAB = sc.tile([128, 2, 512], F32, tag="sc", name=f"s01_{j}_{kb}")
            nc.tensor.matmul(psAB[:, 0, 0:w], kTp[0:64, 128 * kb:128 * kb + 128],
                             qTp[0:64, qs:qs + w], start=True, stop=True)
            nc.tensor.matmul(psAB[:, 1, 0:w], kTp[64:128, 128 * kb:128 * kb + 128],
                             qTp[64:128, qs:qs + w], start=True, stop=True)
            ew = work.tile([128, 2, 512], BF16, tag="ew", bufs=4,
                           name=f"e01_{j}_{kb}")
            nc.scalar.activation(ew[:, :, 0:w], psAB[:, :, 0:w], AF.Exp)
            if kb >= 4 * j:     # diagonal block: zero upper triangle
                nc.vector.tensor_mul(ew[:, :, 0:128], ew[:, :, 0:128], tri2_t)
            for h in range(2):
                nc.tensor.matmul(pn01[:, h, w0:512], v_t[:, kb, h, :],
                                 ew[:, h, 0:w],
                                 start=(kb == 0), stop=(kb == 4 * j + 3))

        # head 2: two key blocks per step via the replicated q2/k2 copies
        def h2_block(t):
            kbA, kbB = 2 * t, 2 * t + 1
            w0 = max(0, 256 * t - 512 * j)
            w = 512 - w0
            qs = 512 * j + w0
            psAB = sc.tile([128, 2, 512], F32, tag="sc", name=f"s2_{j}_{t}")
            nc.tensor.matmul(psAB[:, 0, 0:w], k2rep[0:64, 128 * kbA:128 * kbA + 128],
                             q2rep[0:64, qs:qs + w], start=True, stop=True)
            nc.tensor.matmul(psAB[:, 1, 0:w], k2rep[64:128, 128 * kbB:128 * kbB + 128],
                             q2rep[64:128, qs:qs + w], start=True, stop=True)
            ew = work.tile([128, 2, 512], BF16, tag="ew", bufs=4,
                           name=f"e2_{j}_{t}")
            nc.scalar.activation(ew[:, :, 0:w], psAB[:, :, 0:w], AF.Exp)
            if t >= 2 * j:      # diagonal pair: [tri|1] on A, [0|tri] on B
                nc.vector.tensor_mul(ew[:, :, 0:256], ew[:, :, 0:256], mh2_t)
            nc.tensor.matmul(pn2[:, w0:512], v_t[:, kbA, 2, :], ew[:, 0, 0:w],
                             start=(t == 0), stop=False)
            nc.tensor.matmul(pn2[:, w0:512], v_t[:, kbB, 2, :], ew[:, 1, 0:w],
                             start=False, stop=(t == 2 * j + 1))

        # h01 phase, then h2 phase; norm01 overlaps the h2 phase
        for kb in range(4 * j + 4):
            h01_block(kb)
            fill()
        # normalize heads 0/1 (DVE: copy den out of PSUM, approx-recip, mul)
        den01 = work.tile([64, 2, 512], F32, tag="den01", name=f"den01_{j}")
        nc.vector.tensor_copy(den01, pn01[64:128, :, :])
        rden01 = work.tile([64, 2, 512], F32, tag="rden01", name=f"rden01_{j}")
        nc.vector.reciprocal_approx_fast(out=rden01, in_=den01)
        nc.vector.tensor_mul(yTall[:, 0:2, jsl], pn01[0:64, :, :], rden01)
        # publish the h01 rows early; the collective waits for both parts
        nc.sync.dma_start(
            out=ag_ins[j].rearrange("(h p) q -> p h q", p=64)[:, 0:2],
            in_=yTall[:, 0:2, jsl])
        for t in range(2 * j + 2):
            h2_block(t)
            fill()
        # normalize h2
        den2 = work.tile([64, 512], F32, tag="den2", name=f"den2_{j}")
        nc.vector.tensor_copy(den2, pn2[64:128, :])
        rden2 = work.tile([64, 512], F32, tag="rden2", name=f"rden2_{j}")
        nc.vector.reciprocal_approx_fast(out=rden2, in_=den2)
        nc.vector.tensor_mul(yTall[:, 2, jsl], pn2[0:64, :], rden2)
        # publish the h2 rows (h01 rows were published mid-chunk)
        nc.sync.dma_start(
            out=ag_ins[j].rearrange("(h p) q -> p h q", p=64)[:, 2],
            in_=yTall[:, 2, jsl])
        if with_collective:
            nc.gpsimd.collective_compute(
                "AllGather", mybir.AluOpType.bypass, replica_groups=GROUPS,
                ins=[ag_ins[j].opt()], outs=[ag_outs[j].opt()])
        # drain remaining fillers
        for f in fit:
            f()

    # ---- schedule: chunks 0..3; projections for chunk j+1 and the
    # out-projection of chunk j-1 are interleaved into chunk j's blocks ----
    # startup: chunk-0 q/k slots + v kb0..3 (v on the "sc" ring so the two
    # startup streams use independent PSUM banks)
    # startup: chunk-0 q/k slots + v kb0..3, ALL on the po ring so the sc
    # ring stays free for chunk-0 score pipelining (v tiles squatting in the
    # sc ring stalled the first score blocks ~5us)
    qkproj_unit(0, 1)
    qkproj_unit(0, 0)
    vproj_unit(0, in_sc=True)
    qkproj_unit(0, 2)
    vproj_unit(1, in_sc=True)
    vproj_unit(2, in_sc=True)
    vproj_unit(3, in_sc=True)

    def next_chunk_fillers(j):
        # projections needed by chunk j+1, then outproj of chunk j-1
        # (slot 2 first: its chain is longest - adds + replication DMAs)
        f = []
        if j + 1 < QC:
            f.append(lambda s=1: qkproj_unit(j + 1, 1))
            f.append(lambda lc=4 * j + 4: vproj_unit(lc))
            f.append(lambda s=0: qkproj_unit(j + 1, 0))
            f.append(lambda lc=4 * j + 5: vproj_unit(lc))
            f.append(lambda s=2: qkproj_unit(j + 1, 2))
            if j + 1 < QC - 1:
                # chunk j+1 < 3: its late-kb v tiles are produced here
                f.append(lambda lc=4 * j + 6: vproj_unit(lc))
                f.append(lambda lc=4 * j + 7: vproj_unit(lc))
        if j == QC - 1:
            # chunk 3: the last v tiles (needed only by its own late blocks)
            f.append(lambda: vproj_unit(4 * j + 2))
            f.append(lambda: vproj_unit(4 * j + 3))
        if j - 1 >= 0:
            f.extend(outproj_units(j - 1))
        return f

    for j in range(QC):
        f = next_chunk_fillers(j)
        if j == 0 and pending_tail:
            # previous iteration's deferred tail out-projection: appended
            # AFTER this iteration's projection fillers so a slow AllGather
            # can't head-of-line-block them on the in-order PE queue
            f = f + list(pending_tail)
        attn_chunk(j, f)
    tail = outproj_units(QC - 1)
    if defer_tail:
        return tail
    for u in tail:
        u()
    return None


def build_nc(n_iters=1, with_collective=True, unroll=False):
    nc = bacc.Bacc("TRN2", target_bir_lowering=False, debug=False, num_devices=NC)
    io = (
        nc.declare_dram_parameter("xT", [E, L], BF16, isOutput=False),
        nc.declare_dram_parameter("wqk", [128, 3, EC, 128], BF16, isOutput=False),
        nc.declare_dram_parameter("wv", [128, EC, 192], BF16, isOutput=False),
        nc.declare_dram_parameter("wo", [128, H // 2, 192], BF16, isOutput=False),
        nc.declare_dram_parameter("cbf", [128, 768], BF16, isOutput=False),
        nc.declare_dram_parameter("cf32", [128, 197], F32, isOutput=False),
        nc.declare_dram_parameter("out_bt", [192, L], BF16, isOutput=True),
    )
    with tile.TileContext(nc) as tc:
        with (
            tc.tile_pool(name="consts", bufs=1) as consts,
            tc.tile_pool(name="pers", bufs=1) as pers,
            tc.tile_pool(name="work", bufs=3) as work,
            tc.tile_pool(name="sc", bufs=2, space="PSUM") as sc,
            tc.tile_pool(name="num", bufs=1, space="PSUM") as num,
            tc.tile_pool(name="po", bufs=1, space="PSUM") as po,
            tc.tile_pool(name="dram", bufs=1, space="DRAM") as dram,
        ):
            pools = (consts, pers, work, sc, num, po, dram)
            pers_tiles = _make_pers(nc, pers, io)
            if n_iters == 1:
                _emit_body(nc, tc, io, pools, pers_tiles, with_collective)
            elif unroll:
                pending = None
                for it in range(n_iters):
                    pending = _emit_body(nc, tc, io, pools, pers_tiles,
                                         with_collective,
                                         defer_tail=(it < n_iters - 1),
                                         pending_tail=pending)
            else:
                with tc.For_i(0, n_iters, 1):
                    _emit_body(nc, tc, io, pools, pers_tiles, with_collective)
    nc.finalize()
    return nc


# ---------------------------------------------------------------------------
def prep_in_maps(x, Wqkv, bqkv, Wo, bo):
    import ml_dtypes
    BF = ml_dtypes.bfloat16
    x = np.asarray(x, np.float32)
    Wqkv = np.asarray(Wqkv, np.float32)
    bqkv = np.asarray(bqkv, np.float32)
    Wo = np.asarray(Wo, np.float32)
    bo = np.asarray(bo, np.float32)

    tri01 = np.triu(np.ones((128, 128), np.float32))          # keep q >= k
    tri2 = np.stack([tri01, tri01], axis=1).astype(BF)        # [128, 2, 128]
    maskh2 = np.zeros((128, 2, 256), np.float32)
    maskh2[:, 0, 0:128] = tri01
    maskh2[:, 0, 128:256] = 1.0
    maskh2[:, 1, 128:256] = tri01
    maskh2 = maskh2.astype(BF)

    in_maps = []
    for c in range(NC):
        b, rank = divmod(c, 4)
        heads = [HPC * rank + i for i in range(HPC)]
        g0, g1, g2 = heads

        def qcol(g):
            return Wqkv[:, g * 192:g * 192 + 64] / 8.0

        def kcol(g):
            return Wqkv[:, g * 192 + 64:g * 192 + 128]

        def vcol(g):
            return Wqkv[:, g * 192 + 128:g * 192 + 192]

        wqk = np.zeros((3, E, 128), np.float32)
        wqk[0] = np.concatenate([qcol(g0), qcol(g1)], axis=1)
        wqk[1] = np.concatenate([kcol(g0), kcol(g1)], axis=1)
        wqk[2] = np.concatenate([qcol(g2), kcol(g2)], axis=1)

        bqk = np.zeros((128, 3), np.float32)
        bqk[0:64, 0] = bqkv[g0 * 192:g0 * 192 + 64] / 8.0
        bqk[64:128, 0] = bqkv[g1 * 192:g1 * 192 + 64] / 8.0
        bqk[0:64, 1] = bqkv[g0 * 192 + 64:g0 * 192 + 128]
        bqk[64:128, 1] = bqkv[g1 * 192 + 64:g1 * 192 + 128]
        bqk[0:64, 2] = bqkv[g2 * 192:g2 * 192 + 64] / 8.0
        bqk[64:128, 2] = bqkv[g2 * 192 + 64:g2 * 192 + 128]

        wv = np.concatenate([vcol(g) for g in heads], axis=1)      # [768, 192]
        bv_row = np.concatenate(
            [bqkv[g * 192 + 128:g * 192 + 192] for g in heads])
        bv = np.broadcast_to(bv_row, (128, 192)).copy()

        # head-pair layout: wo[d, p, e'] = Wo[128*p+d, 192*rank+e']
        # (partitions 0:64 = head 2p, 64:128 = head 2p+1)
        wo = np.ascontiguousarray(
            Wo.reshape(H // 2, 128, E)[:, :, 192 * rank:192 * rank + 192]
            .transpose(1, 0, 2)).astype(BF)
        bo_s = bo[192 * rank:192 * rank + 192].reshape(192, 1)

        wqk_p = np.ascontiguousarray(
            wqk.reshape(3, EC, 128, 128).transpose(2, 0, 1, 3))
        wv_p = np.ascontiguousarray(
            wv.reshape(EC, 128, 192).transpose(1, 0, 2))
        # packed const blobs: cbf = tri2 | mh2 (bf16), cf32 = bqk|bv|bo1|bo2
        cbf = np.concatenate([tri2.reshape(128, 256),
                              maskh2.reshape(128, 512)], axis=1)
        cf32 = np.zeros((128, 197), np.float32)
        cf32[:, 0:3] = bqk
        cf32[:, 3:195] = bv
        cf32[:, 195] = bo_s[0:128, 0]
        cf32[0:64, 196] = bo_s[128:192, 0]
        in_maps.append({
            "xT": np.ascontiguousarray(x[b].T).astype(BF),
            "wqk": wqk_p.astype(BF),
            "wv": wv_p.astype(BF),
            "wo": wo,
            "cbf": np.ascontiguousarray(cbf),
            "cf32": cf32,
        })
    return in_maps


def assemble(results):
    out = np.zeros((B, L, E), np.float32)
    for b in range(B):
        cols = np.concatenate(
            [np.asarray(results[4 * b + r]["out_bt"], np.float32)
             for r in range(4)], axis=0)       # [768, L]
        out[b] = cols.T
    return out


_NC_CACHE = {}


def _get_nc(n_iters=1):
    if n_iters not in _NC_CACHE:
        _NC_CACHE[n_iters] = build_nc(n_iters)
    return _NC_CACHE[n_iters]


def kernel(x, Wqkv, bqkv, Wo, bo, train=0, **_unused):
    nc = _get_nc(1)
    in_maps = prep_in_maps(x, Wqkv, bqkv, Wo, bo)
    res = bass_utils.run_bass_kernel_spmd(nc, in_maps, core_ids=list(range(NC)))
    return assemble(res.results)



# revision 124
# speedup vs baseline: 1.3930x; 1.3930x over previous
"""Causal self-attention (B=2, L=2048, E=768, H=12) on 8 trn2 NeuronCores.

Sharding: data parallel over B (cores 0-3 -> b=0, cores 4-7 -> b=1), tensor
parallel over heads (each core owns 3 heads).  Per core:
  - all-bf16 operands; q/k projections in transposed [d, L] layout,
  - scores kept TRANSPOSED S^T [keys, queries]; heads 0/1 as two K=64 PE
    row-tile matmuls (partition bases 0/64); head 2 pairs adjacent key
    blocks the same way using replicated q2/k2 copies,
  - numerator stationary is [V_h | ones] so PSUM rows 64:128 accumulate the
    softmax denominator replicated across 64 partitions; normalization is
    copy + fast approx-reciprocal + multiply on DVE,
  - causal diagonal masking via a bf16 multiply on the exp output (DVE),
  - query chunks processed 0..3; the attention stream (scores -> exp ->
    mask -> numerator) is ACT-limited, so ALL projection work is emitted as
    interleaved PE "filler" units between attention blocks: chunk j carries
    the q/k/v projections of chunk j+1 and the out-projection of chunk j-1,
  - per chunk: y (3 heads) is published (h01 rows early, right after their
    normalize) and AllGathered (bf16, 192KB -> 768KB, one per chunk on the
    otherwise-empty gpsimd queue); every core computes the 12-head
    out-projection for its 192 output columns as 6 K=128 head-PAIR matmuls
    (+64-col half), tanh+bias on ACT, bf16 output,
  - PSUM: scores 2x[128,2,512] (4 banks) + numerators (3) + filler/outproj
    ring (1); startup wire order: wqk slots 0/1 + chunk-0 x first,
  - persistent tiles (q/k/v/y, the 'ones' memset, and the load-once consts
    cf32 + wo) are hoisted out of the body; in unrolled builds each
    iteration's tail out-projection is DEFERRED into the next iteration's
    chunk-0 filler drain so a pending AllGather never head-of-line-blocks
    the next iteration's projections on the in-order PE queue (requires the
    deferred units to read only load-once tiles, else reload-WARs deadlock
    the ACT queue).
HW timing (unrolled-with-collectives interleaved slope): 80-112 us/iter
depending on device contention (best sample 80.2 us ~= the 78 us PE matmul
roofline); TimelineSim (no-collective single body): 110.5 us.
"""
import hashlib
import os
import shutil

import numpy as np

import concourse.bacc as bacc
import concourse.mybir as mybir
import concourse.tile as tile
from concourse import bass_utils, bass2jax

F32 = mybir.dt.float32
BF16 = mybir.dt.bfloat16
AF = mybir.ActivationFunctionType

B, L, E, H, D = 2, 2048, 768, 12, 64
HPC = 3                      # heads per core
NC = 8
GROUPS = [[0, 1, 2, 3], [4, 5, 6, 7]]
EC = E // 128                # 6 embedding chunks
QC = L // 512                # 4 query chunks of 512
KB = L // 128                # 16 key blocks of 128

# ---------------------------------------------------------------------------
# NEFF compile memoization (same BIR -> same NEFF); safe, process-local.
_orig_compile = bass_utils.compile_bir_kernel
_CACHE_DIR = os.environ.get("NEFF_MEMO_DIR", "/tmp/neff_cache")


def _memo_compile(bir_json, tmpdir, neff_name="file.neff"):
    try:
        os.makedirs(_CACHE_DIR, exist_ok=True)
        key = hashlib.sha256(bir_json).hexdigest()[:24]
        cached = os.path.join(_CACHE_DIR, f"{key}.neff")
        if os.path.exists(cached):
            dst = os.path.join(tmpdir, neff_name)
            shutil.copy(cached, dst)
            return dst
        path = _orig_compile(bir_json, tmpdir, neff_name)
        shutil.copy(path, cached)
        return path
    except OSError:
        return _orig_compile(bir_json, tmpdir, neff_name)


bass_utils.compile_bir_kernel = _memo_compile
bass2jax.compile_bir_kernel = _memo_compile


# ---------------------------------------------------------------------------
def _make_pers(nc, pers, io):
    """Persistent tiles + iteration-invariant consts, allocated/loaded once
    per NEFF.  wo_t and cf32 MUST be load-once: a per-iteration reload would
    WAR against the deferred tail out-projection's reads and deadlock the
    in-order ACT queue (reload -> deferred tanh -> behind next exps)."""
    (xT, wqk, wv, wo, cbf_d, cf32_d, out_bt) = io
    qTp = pers.tile([128, L], BF16, name="qTp")    # h0 rows 0:64, h1 64:128
    kTp = pers.tile([128, L], BF16, name="kTp")
    q2rep = pers.tile([128, L], BF16, name="q2rep")  # h2 q replicated
    k2rep = pers.tile([128, L], BF16, name="k2rep")  # h2 k replicated
    v_t = pers.tile([128, KB, HPC, 128], BF16, name="v_t")  # [keys,kb,h,v|1]
    yTall = pers.tile([64, HPC, L], BF16, name="yTall")
    # once per NEFF, at the head of the gpsimd queue (before any collective
    # is enqueued, so nothing blocks); the Pool engine's DMA path runs in
    # parallel with the HWDGE wire, keeping these off the startup x path
    nc.gpsimd.memset(v_t[:, :, :, 64:128], 1.0)
    cf32 = pers.tile([128, 197], F32, name="cf32")  # bqk|bv|bo1|bo2
    nc.gpsimd.dma_start(out=cf32, in_=cf32_d.ap())
    wo_t = pers.tile([128, H // 2, 192], BF16, name="wo_t")
    nc.gpsimd.dma_start(out=wo_t, in_=wo.ap())
    return (qTp, kTp, q2rep, k2rep, v_t, yTall, cf32, wo_t)


def _emit_body(nc, tc, io, pools, pers_tiles, with_collective=True,
               defer_tail=False, pending_tail=None):
    (xT, wqk, wv, wo, cbf_d, cf32_d, out_bt) = io
    consts, pers, work, sc, num, po, dram = pools

    # ---- constant loads -------------------------------------------------
    # wire-order startup: wqk + chunk-0 x columns first (they gate the first
    # projections), then the small const blobs, then the rest of x.  The
    # gpsimd queue carries ONLY the collectives so an in-flight AllGather
    # never blocks the next iteration's loads.
    wqk_t = consts.tile([128, 3, EC, 128], BF16, name="wqk_t")
    nc.scalar.dma_start(out=wqk_t[:, 0:2], in_=wqk.ap()[:, 0:2])
    xt_t = consts.tile([128, EC, L], BF16, name="xt_t")
    xT_r = xT.ap().rearrange("(c p) m -> p c m", p=128)
    qs_eng = [nc.sync, nc.scalar]
    for c in range(EC):
        qs_eng[c % 2].dma_start(out=xt_t[:, c, 0:512], in_=xT_r[:, c, 0:512])
    nc.scalar.dma_start(out=wqk_t[:, 2:3], in_=wqk.ap()[:, 2:3])
    cbf = consts.tile([128, 768], BF16, name="cbf")   # tri2 | mh2
    nc.sync.dma_start(out=cbf, in_=cbf_d.ap())
    wv_t = consts.tile([128, EC, 192], BF16, name="wv_t")
    nc.scalar.dma_start(out=wv_t, in_=wv.ap())
    tri2_t = cbf[:, 0:256].rearrange("p (a b) -> p a b", a=2)
    mh2_t = cbf[:, 256:768].rearrange("p (a b) -> p a b", a=2)
    qi = 0
    for jj in range(1, QC):
        jsl = slice(512 * jj, 512 * jj + 512)
        for c in range(EC):
            qs_eng[qi % 2].dma_start(out=xt_t[:, c, jsl], in_=xT_r[:, c, jsl])
            qi += 1

    # ---- persistent tiles + load-once consts (shared across iterations) --
    (qTp, kTp, q2rep, k2rep, v_t, yTall, cf32, wo_t) = pers_tiles
    bqk_t = cf32[:, 0:3]
    bv_t = cf32[:, 3:195]
    bo1_t = cf32[:, 195:196]
    bo2_t = cf32[0:64, 196:197]

    ag_ins = [dram.tile([192, 512], BF16, name=f"ag_in{j}") for j in range(QC)]
    ag_outs = [dram.tile([768, 512], BF16, name=f"ag_out{j}")
               for j in range(QC)]

    # ---- projection / out-projection units (emitted interleaved) --------
    # slot 0 = [Wq_h0|Wq_h1], slot 1 = [Wk_h0|Wk_h1], slot 2 = [Wq_h2|Wk_h2]
    def _qk_epilogue(jj, slot, ps):
        jsl = slice(512 * jj, 512 * jj + 512)
        if slot == 0:
            nc.vector.tensor_scalar_add(out=qTp[:, jsl], in0=ps,
                                        scalar1=bqk_t[:, 0:1])
        elif slot == 1:
            nc.vector.tensor_scalar_add(out=kTp[:, jsl], in0=ps,
                                        scalar1=bqk_t[:, 1:2])
        else:
            nc.vector.tensor_scalar_add(out=q2rep[0:64, jsl], in0=ps[0:64],
                                        scalar1=bqk_t[0:64, 2:3])
            nc.vector.tensor_scalar_add(out=k2rep[64:128, jsl],
                                        in0=ps[64:128],
                                        scalar1=bqk_t[64:128, 2:3])
            nc.sync.dma_start(out=q2rep[64:128, jsl], in_=q2rep[0:64, jsl])
            nc.sync.dma_start(out=k2rep[0:64, jsl], in_=k2rep[64:128, jsl])

    def qkproj_unit(jj, slot, tag="po"):
        jsl = slice(512 * jj, 512 * jj + 512)
        ps = po.tile([128, 512], F32, tag=tag, name=f"ps_qk{slot}_{jj}")
        for c in range(EC):
            nc.tensor.matmul(ps, wqk_t[:, slot, c], xt_t[:, c, jsl],
                             start=(c == 0), stop=(c == EC - 1))
        _qk_epilogue(jj, slot, ps)

    def vproj_unit(lc, in_sc=False):
        pool_, tag_ = (sc, "sc") if in_sc else (po, "po")
        ps = pool_.tile([128, 192], F32, tag=tag_, name=f"ps_v{lc}")
        for c in range(EC):
            nc.tensor.matmul(ps, xt_t[:, c, 128 * lc:128 * lc + 128], wv_t[:, c],
                             start=(c == 0), stop=(c == EC - 1))
        nc.vector.tensor_add(v_t[:, lc, :, 0:64],
                             ps.rearrange("p (h d) -> p h d", h=HPC),
                             bv_t.rearrange("p (h d) -> p h d", h=HPC))

    def outproj_units(j):
        # head-PAIR layout: yAll [128, 6, 512], partitions 0:64 = head 2p,
        # 64:128 = head 2p+1 -> full K=128 contraction per matmul.
        jsl = slice(512 * j, 512 * j + 512)
        yAll = work.tile([128, H // 2, 512], BF16, tag="yall", name=f"yAll{j}")
        po_t = [None, None]
        src = ag_outs[j].rearrange("(hp p) q -> p hp q", p=128)

        def u_load():
            # per-pair DMAs (sync queue) so out-proj matmuls pipeline with
            # the gather read; the no-collective fallback mimics the same 6
            # DRAM reads (first one depends on the publish, as the AG would)
            if with_collective:
                for p in range(H // 2):
                    nc.sync.dma_start(out=yAll[:, p, :], in_=src[:, p, :])
            else:
                nc.sync.dma_start(out=yAll[:, 0, :], in_=ag_ins[j][0:128, :])
                for p in range(1, H // 2):
                    nc.sync.dma_start(out=yAll[:, p, :], in_=src[:, p, :])

        def u_mm1():
            po_t[0] = po.tile([128, 512], F32, tag="po", name=f"po1_{j}")
            for p in range(H // 2):
                nc.tensor.matmul(po_t[0], wo_t[:, p, 0:128], yAll[:, p, :],
                                 start=(p == 0), stop=(p == H // 2 - 1))

        def u_act1():
            t1 = work.tile([128, 512], BF16, tag="t1", name=f"t1_{j}")
            nc.scalar.activation(t1, po_t[0], AF.Tanh, bias=bo1_t, scale=1.0)
            nc.sync.dma_start(out=out_bt.ap()[0:128, jsl], in_=t1)

        def u_mm2():
            po_t[1] = po.tile([64, 512], F32, tag="po", name=f"po2_{j}")
            for p in range(H // 2):
                nc.tensor.matmul(po_t[1], wo_t[:, p, 128:192], yAll[:, p, :],
                                 start=(p == 0), stop=(p == H // 2 - 1))

        def u_act2():
            t2 = work.tile([64, 512], BF16, tag="t2", name=f"t2_{j}")
            nc.scalar.activation(t2, po_t[1], AF.Tanh, bias=bo2_t, scale=1.0)
            nc.scalar.dma_start(out=out_bt.ap()[128:192, jsl], in_=t2)

        return [u_load, u_mm1, u_act1, u_mm2, u_act2]

    # ---- attention chunk with interleaved filler units ------------------
    def attn_chunk(j, fillers=()):
        fit = iter(fillers)

        def fill():
            f = next(fit, None)
            if f is not None:
                f()

        jsl = slice(512 * j, 512 * j + 512)
        pn01 = num.tile([128, 2, 512], F32, tag="pn01", name=f"pn01_{j}")
        pn2 = num.tile([128, 512], F32, tag="pn2", name=f"pn2_{j}")

        # heads 0/1: one key block per step, two concurrent row tiles
        def h01_block(kb):
            w0 = max(0, 128 * kb - 512 * j)
            w = 512 - w0
            qs = 512 * j + w0
            psAB = sc.tile([128, 2, 512], F32, tag="sc", name=f"s01_{j}_{kb}")
            nc.tensor.matmul(psAB[:, 0, 0:w], kTp[0:64, 128 * kb:128 * kb + 128],
                             qTp[0:64, qs:qs + w], start=True, stop=True)
            nc.tensor.matmul(psAB[:, 1, 0:w], kTp[64:128, 128 * kb:128 * kb + 128],
                             qTp[64:128, qs:qs + w], start=True, stop=True)
            ew = work.tile([128, 2, 512], BF16, tag="ew", bufs=4,
                           name=f"e01_{j}_{kb}")
            nc.scalar.activation(ew[:, :, 0:w], psAB[:, :, 0:w], AF.Exp)
            if kb >= 4 * j:     # diagonal block: zero upper triangle
                nc.vector.tensor_mul(ew[:, :, 0:128], ew[:, :, 0:128], tri2_t)
            for h in range(2):
                nc.tensor.matmul(pn01[:, h, w0:512], v_t[:, kb, h, :],
                                 ew[:, h, 0:w],
                                 start=(kb == 0), stop=(kb == 4 * j + 3))

        # head 2: two key blocks per step via the replicated q2/k2 copies
        def h2_block(t):
            kbA, kbB = 2 * t, 2 * t + 1
            w0 = max(0, 256 * t - 512 * j)
            w = 512 - w0
            qs = 512 * j + w0
            psAB = sc.tile([128, 2, 512], F32, tag="sc", name=f"s2_{j}_{t}")
            nc.tensor.matmul(psAB[:, 0, 0:w], k2rep[0:64, 128 * kbA:128 * kbA + 128],
                             q2rep[0:64, qs:qs + w], start=True, stop=True)
            nc.tensor.matmul(psAB[:, 1, 0:w], k2rep[64:128, 128 * kbB:128 * kbB + 128],
                             q2rep[64:128, qs:qs + w], start=True, stop=True)
            ew = work.tile([128, 2, 512], BF16, tag="ew", bufs=4,
                           name=f"e2_{j}_{t}")
            nc.scalar.activation(ew[:, :, 0:w], psAB[:, :, 0:w], AF.Exp)
            if t >= 2 * j:      # diagonal pair: [tri|1] on A, [0|tri] on B
                nc.vector.tensor_mul(ew[:, :, 0:256], ew[:, :, 0:256], mh2_t)
            nc.tensor.matmul(pn2[:, w0:512], v_t[:, kbA, 2, :], ew[:, 0, 0:w],
                             start=(t == 0), stop=False)
            nc.tensor.matmul(pn2[:, w0:512], v_t[:, kbB, 2, :], ew[:, 1, 0:w],
                             start=False, stop=(t == 2 * j + 1))

        # h01 phase, then h2 phase; norm01 overlaps the h2 phase
        for kb in range(4 * j + 4):
            h01_block(kb)
            fill()
        # normalize heads 0/1 (DVE: copy den out of PSUM, approx-recip, mul)
        den01 = work.tile([64, 2, 512], F32, tag="den01", name=f"den01_{j}")
        nc.vector.tensor_copy(den01, pn01[64:128, :, :])
        rden01 = work.tile([64, 2, 512], F32, tag="rden01", name=f"rden01_{j}")
        nc.vector.reciprocal_approx_fast(out=rden01, in_=den01)
        nc.vector.tensor_mul(yTall[:, 0:2, jsl], pn01[0:64, :, :], rden01)
        # publish the h01 rows early; the collective waits for both parts
        nc.sync.dma_start(
            out=ag_ins[j].rearrange("(h p) q -> p h q", p=64)[:, 0:2],
            in_=yTall[:, 0:2, jsl])
        for t in range(2 * j + 2):
            h2_block(t)
            fill()
        # normalize h2
        den2 = work.tile([64, 512], F32, tag="den2", name=f"den2_{j}")
        nc.vector.tensor_copy(den2, pn2[64:128, :])
        rden2 = work.tile([64, 512], F32, tag="rden2", name=f"rden2_{j}")
        nc.vector.reciprocal_approx_fast(out=rden2, in_=den2)
        nc.vector.tensor_mul(yTall[:, 2, jsl], pn2[0:64, :], rden2)
        # publish the h2 rows (h01 rows were published mid-chunk)
        nc.sync.dma_start(
            out=ag_ins[j].rearrange("(h p) q -> p h q", p=64)[:, 2],
            in_=yTall[:, 2, jsl])
        if with_collective:
            nc.gpsimd.collective_compute(
                "AllGather", mybir.AluOpType.bypass, replica_groups=GROUPS,
                ins=[ag_ins[j].opt()], outs=[ag_outs[j].opt()])
        # drain remaining fillers
        for f in fit:
            f()

    # ---- schedule: chunks 0..3; projections for chunk j+1 and the
    # out-projection of chunk j-1 are interleaved into chunk j's blocks ----
    # startup: chunk-0 q/k slots + v kb0..3 (v on the "sc" ring so the two
    # startup streams use independent PSUM banks)
    # startup: chunk-0 q/k slots + v kb0..3, ALL on the po ring so the sc
    # ring stays free for chunk-0 score pipelining (v tiles squatting in the
    # sc ring stalled the first score blocks ~5us)
    qkproj_unit(0, 1)
    qkproj_unit(0, 0)
    vproj_unit(0, in_sc=True)
    qkproj_unit(0, 2)
    vproj_unit(1, in_sc=True)
    vproj_unit(2, in_sc=True)
    vproj_unit(3, in_sc=True)

    def next_chunk_fillers(j):
        # projections needed by chunk j+1, then outproj of chunk j-1
        # (slot 2 first: its chain is longest - adds + replication DMAs)
        f = []
        if j + 1 < QC:
            f.append(lambda s=1: qkproj_unit(j + 1, 1))
            f.append(lambda lc=4 * j + 4: vproj_unit(lc))
            f.append(lambda s=0: qkproj_unit(j + 1, 0))
            f.append(lambda lc=4 * j + 5: vproj_unit(lc))
            f.append(lambda s=2: qkproj_unit(j + 1, 2))
            if j + 1 < QC - 1:
                # chunk j+1 < 3: its late-kb v tiles are produced here
                f.append(lambda lc=4 * j + 6: vproj_unit(lc))
                f.append(lambda lc=4 * j + 7: vproj_unit(lc))
        if j == QC - 1:
            # chunk 3: the last v tiles (needed only by its own late blocks)
            f.append(lambda: vproj_unit(4 * j + 2))
            f.append(lambda: vproj_unit(4 * j + 3))
        if j - 1 >= 0:
            f.extend(outproj_units(j - 1))
        return f

    for j in range(QC):
        f = next_chunk_fillers(j)
        if j == 0 and pending_tail:
            # previous iteration's deferred tail out-projection: appended
            # AFTER this iteration's projection fillers so a slow AllGather
            # can't head-of-line-block them on the in-order PE queue
            f = f + list(pending_tail)
        attn_chunk(j, f)
    tail = outproj_units(QC - 1)
    if defer_tail:
        return tail
    for u in tail:
        u()
    return None


def build_nc(n_iters=1, with_collective=True, unroll=False):
    nc = bacc.Bacc("TRN2", target_bir_lowering=False, debug=False, num_devices=NC)
    io = (
        nc.declare_dram_parameter("xT", [E, L], BF16, isOutput=False),
        nc.declare_dram_parameter("wqk", [128, 3, EC, 128], BF16, isOutput=False),
        nc.declare_dram_parameter("wv", [128, EC, 192], BF16, isOutput=False),
        nc.declare_dram_parameter("wo", [128, H // 2, 192], BF16, isOutput=False),
        nc.declare_dram_parameter("cbf", [128, 768], BF16, isOutput=False),
        nc.declare_dram_parameter("cf32", [128, 197], F32, isOutput=False),
        nc.declare_dram_parameter("out_bt", [192, L], BF16, isOutput=True),
    )
    with tile.TileContext(nc) as tc:
        with (
            tc.tile_pool(name="consts", bufs=1) as consts,
            tc.tile_pool(name="pers", bufs=1) as pers,
            tc.tile_pool(name="work", bufs=3) as work,
            tc.tile_pool(name="sc", bufs=2, space="PSUM") as sc,
            tc.tile_pool(name="num", bufs=1, space="PSUM") as num,
            tc.tile_pool(name="po", bufs=1, space="PSUM") as po,
            tc.tile_pool(name="dram", bufs=1, space="DRAM") as dram,
        ):
            pools = (consts, pers, work, sc, num, po, dram)
            pers_tiles = _make_pers(nc, pers, io)
            if n_iters == 1:
                _emit_body(nc, tc, io, pools, pers_tiles, with_collective)
            elif unroll:
                pending = None
                for it in range(n_iters):
                    pending = _emit_body(nc, tc, io, pools, pers_tiles,
                                         with_collective,
                                         defer_tail=(it < n_iters - 1),
                                         pending_tail=pending)
            else:
                with tc.For_i(0, n_iters, 1):
                    _emit_body(nc, tc, io, pools, pers_tiles, with_collective)
    nc.finalize()
    return nc


# ---------------------------------------------------------------------------
def prep_in_maps(x, Wqkv, bqkv, Wo, bo):
    import ml_dtypes
    BF = ml_dtypes.bfloat16
    x = np.asarray(x, np.float32)
    Wqkv = np.asarray(Wqkv, np.float32)
    bqkv = np.asarray(bqkv, np.float32)
    Wo = np.asarray(Wo, np.float32)
    bo = np.asarray(bo, np.float32)

    tri01 = np.triu(np.ones((128, 128), np.float32))          # keep q >= k
    tri2 = np.stack([tri01, tri01], axis=1).astype(BF)        # [128, 2, 128]
    maskh2 = np.zeros((128, 2, 256), np.float32)
    maskh2[:, 0, 0:128] = tri01
    maskh2[:, 0, 128:256] = 1.0
    maskh2[:, 1, 128:256] = tri01
    maskh2 = maskh2.astype(BF)

    in_maps = []
    for c in range(NC):
        b, rank = divmod(c, 4)
        heads = [HPC * rank + i for i in range(HPC)]
        g0, g1, g2 = heads

        def qcol(g):
            return Wqkv[:, g * 192:g * 192 + 64] / 8.0

        def kcol(g):
            return Wqkv[:, g * 192 + 64:g * 192 + 128]

        def vcol(g):
            return Wqkv[:, g * 192 + 128:g * 192 + 192]

        wqk = np.zeros((3, E, 128), np.float32)
        wqk[0] = np.concatenate([qcol(g0), qcol(g1)], axis=1)
        wqk[1] = np.concatenate([kcol(g0), kcol(g1)], axis=1)
        wqk[2] = np.concatenate([qcol(g2), kcol(g2)], axis=1)

        bqk = np.zeros((128, 3), np.float32)
        bqk[0:64, 0] = bqkv[g0 * 192:g0 * 192 + 64] / 8.0
        bqk[64:128, 0] = bqkv[g1 * 192:g1 * 192 + 64] / 8.0
        bqk[0:64, 1] = bqkv[g0 * 192 + 64:g0 * 192 + 128]
        bqk[64:128, 1] = bqkv[g1 * 192 + 64:g1 * 192 + 128]
        bqk[0:64, 2] = bqkv[g2 * 192:g2 * 192 + 64] / 8.0
        bqk[64:128, 2] = bqkv[g2 * 192 + 64:g2 * 192 + 128]

        wv = np.concatenate([vcol(g) for g in heads], axis=1)      # [768, 192]
        bv_row = np.concatenate(
            [bqkv[g * 192 + 128:g * 192 + 192] for g in heads])
        bv = np.broadcast_to(bv_row, (128, 192)).copy()

        # head-pair layout: wo[d, p, e'] = Wo[128*p+d, 192*rank+e']
        # (partitions 0:64 = head 2p, 64:128 = head 2p+1)
        wo = np.ascontiguousarray(
            Wo.reshape(H // 2, 128, E)[:, :, 192 * rank:192 * rank + 192]
            .transpose(1, 0, 2)).astype(BF)
        bo_s = bo[192 * rank:192 * rank + 192].reshape(192, 1)

        wqk_p = np.ascontiguousarray(
            wqk.reshape(3, EC, 128, 128).transpose(2, 0, 1, 3))
        wv_p = np.ascontiguousarray(
            wv.reshape(EC, 128, 192).transpose(1, 0, 2))
        # packed const blobs: cbf = tri2 | mh2 (bf16), cf32 = bqk|bv|bo1|bo2
        cbf = np.concatenate([tri2.reshape(128, 256),
                              maskh2.reshape(128, 512)], axis=1)
        cf32 = np.zeros((128, 197), np.float32)
        cf32[:, 0:3] = bqk
        cf32[:, 3:195] = bv
        cf32[:, 195] = bo_s[0:128, 0]
        cf32[0:64, 196] = bo_s[128:192, 0]
        in_maps.append({
            "xT": np.ascontiguousarray(x[b].T).astype(BF),
            "wqk": wqk_p.astype(BF),
            "wv": wv_p.astype(BF),
            "wo": wo,
            "cbf": np.ascontiguousarray(cbf),
            "cf32": cf32,
        })
    return in_maps


def assemble(results):
    out = np.zeros((B, L, E), np.float32)
    for b in range(B):
        cols = np.concatenate(
            [np.asarray(results[4 * b + r]["out_bt"], np.float32)
             for r in range(4)], axis=0)       # [768, L]
        out[b] = cols.T
    return out


_NC_CACHE = {}


def _get_nc(n_iters=1):
    if n_iters not in _NC_CACHE:
        _NC_CACHE[n_iters] = build_nc(n_iters)
    return _NC_CACHE[n_iters]


def kernel(x, Wqkv, bqkv, Wo, bo, train=0, **_unused):
    nc = _get_nc(1)
    in_maps = prep_in_maps(x, Wqkv, bqkv, Wo, bo)
    res = bass_utils.run_bass_kernel_spmd(nc, in_maps, core_ids=list(range(NC)))
    return assemble(res.results)

